# revision 14
# baseline (speedup 1.0000x reference)
"""AttentionSubsample kernel for 8 trn2 NeuronCores (v2.1).

Sharding: head-parallel (8 heads -> 8 cores); final projection sharded by
output channels after an AllGather of per-head attention outputs.

Cost-model-driven design:
- Relative-position bias is added INTO the QK PSUM by a cheap fp8e4m3
  DoubleRow matmul (identity stationary, bias moving; 0.5 cyc/row) instead
  of multiplying exp(bias) on DVE. Softmax scale is folded into the q BN
  affine host-side, so ACT exp reads PSUM directly.
- attn@V runs "swapped": attention chunk stationary, V moving, so the
  moving size is 33 (DV+ones) instead of 448. The attention matrix for one
  (batch, q-chunk) stays resident in SBUF (bf16); the 42-k-tile
  accumulation chains run region-sequential (a start=True clears
  has_written for its whole PSUM bank, so open chains must not interleave
  within a bank; qk regions are 512-aligned so each owns a bank).
- attn@V output is token-major; softmax denominators applied as
  per-partition tensor_scalar scalars; hardswish on DVE; PE transposes
  produce the channel-major hsT for the AllGather + projection.
- DMAs coalesced (packed weights, whole-batch x loads, 6-k-tile bias
  tiles) to limit HWDGE serialization; x is streamed, not front-loaded.
"""

import numpy as np
import ml_dtypes

import concourse.bass as bass
import concourse.mybir as mybir
import concourse.tile as tile
from concourse import bacc
from contextlib import ExitStack
from concourse.bass_utils import run_bass_kernel_spmd

BF16 = mybir.dt.bfloat16
F32 = mybir.dt.float32
F8 = mybir.dt.float8e4
bf16 = ml_dtypes.bfloat16
f8e4 = ml_dtypes.float8_e4m3

B = 2
ROW, COL = 63, 84
ROW_, COL_ = 32, 42
N = ROW * COL            # 5292 kv tokens
NQ = ROW_ * COL_         # 1344 q tokens
NPAD = 5376              # 42*128 padded kv tokens
KTN = NPAD // 128        # 42 k-tiles
QC = 448                 # q chunk
NQC = NQ // QC           # 3
QQ = 112                 # q sub-chunk (attn@V stationary width)
NQQ = QC // QQ           # 4
CIN = 256
H = 8
KD = 16
DV = 32
HKV = KD + DV            # 48 per-head kv channels
KVP = 64                 # padded kv rows: k at 0:16, v at 32:64
OC = 64                  # per-core slice of the 512 output channels
GRP = 2                  # k-tiles per exp group
NGRP = KTN // GRP        # 21
BDMA = 6                 # k-tiles per bias DMA (+1 slop)
NBD = KTN // BDMA        # 7 bias DMAs per q chunk
WTOT = KVP + KD + OC     # 144 packed weight columns
EPS = 1e-5
SCALE = KD ** -0.5
NCORES = 8

LAST_EXEC_NS = None
_prog_cache = {}


def _build_program():
    nc = bacc.Bacc(num_devices=NCORES)

    xT = nc.dram_tensor("xT", [B, 2, 128, NPAD], BF16, kind="ExternalInput")
    xsT = nc.dram_tensor("xsT", [B, 2, 128, NQ], BF16, kind="ExternalInput")
    wT = nc.dram_tensor("wT", [2, 128, WTOT], BF16, kind="ExternalInput")
    gbT = nc.dram_tensor("gbT", [KVP, 6], F32, kind="ExternalInput")
    i2T = nc.dram_tensor("i2T", [128, 2, 128], F8, kind="ExternalInput")
    identT = nc.dram_tensor("identT", [128, 128], BF16, kind="ExternalInput")
    bT = nc.dram_tensor("bT", [NQC, NBD, 128, (BDMA + 1) * QC], F8,
                        kind="ExternalInput")
    yT = nc.dram_tensor("yT", [OC, B * NQ], F32, kind="ExternalOutput")

    with ExitStack() as ctx:
        tc = ctx.enter_context(tile.TileContext(nc))
        const = ctx.enter_context(tc.tile_pool(name="const", bufs=1))
        big = ctx.enter_context(tc.tile_pool(name="big", bufs=1))
        bpool = ctx.enter_context(tc.tile_pool(name="bpool", bufs=3))
        small = ctx.enter_context(tc.tile_pool(name="small", bufs=4))
        drain = ctx.enter_context(tc.tile_pool(name="drain", bufs=3))
        dram = ctx.enter_context(tc.tile_pool(name="dram", bufs=4, space="DRAM"))

        mult = mybir.AluOpType.mult
        add = mybir.AluOpType.add
        amin = mybir.AluOpType.min
        Act = mybir.ActivationFunctionType
        DR = mybir.MatmulPerfMode.DoubleRow

        # ------------------------- consts -------------------------
        w_sb = const.tile([128, 2, WTOT], BF16, tag="w")
        for c in range(2):
            nc.sync.dma_start(out=w_sb[:, c, :], in_=wT[c])
        gb_sb = const.tile([KVP, 6], F32, tag="gb")
        nc.sync.dma_start(out=gb_sb, in_=gbT[:, :])
        i2_sb = const.tile([128, 2, 128], F8, tag="i2")
        nc.sync.dma_start(out=i2_sb, in_=i2T[:, :, :])
        ident_sb = const.tile([128, 128], BF16, tag="ident")
        nc.sync.dma_start(out=ident_sb, in_=identT[:, :])
        eps_t = const.tile([128, 1], F32, tag="eps")
        nc.vector.memset(eps_t, EPS)
        wkv_sb = w_sb[:, :, 0:KVP]
        wq_sb = w_sb[:, :, KVP:KVP + KD]
        wp_sb = w_sb[:, :, KVP + KD:WTOT]
        kvgb_sb = gb_sb[:, 0:2]
        qgb_sb = gb_sb[0:KD, 2:4]
        pgb_sb = gb_sb[0:OC, 4:6]

        # persistent normalized tensors
        kT = big.tile([KD, B, NPAD], BF16, tag="kT")
        qT = big.tile([KD, B, NQ], BF16, tag="qT")
        v_aug = big.tile([128, B, KTN, DV + 1], BF16, tag="vaug")

        def bn_scale_shift(mv, gb, P, name):
            s = small.tile([P, 1], F32, tag=f"s_{name}", name=f"s_{name}")
            t = small.tile([P, 1], F32, tag=f"t_{name}", name=f"t_{name}")
            nc.scalar.activation(out=s, in_=mv[:, 1:2], func=Act.Sqrt,
                                 bias=eps_t[0:P])
            nc.vector.reciprocal(out=s, in_=s)
            nc.vector.tensor_mul(s, s, gb[:, 0:1])
            nc.vector.tensor_mul(t, mv[:, 0:1], s)
            nc.vector.tensor_scalar(out=t, in0=t, scalar1=-1.0, scalar2=None,
                                    op0=mult)
            nc.vector.tensor_add(t, t, gb[:, 1:2])
            return s, t

        # ------------------- start phase (scoped pools) -------------------
        NT_KV = NPAD // QC   # 12
        with ExitStack() as sctx:
            xstr = sctx.enter_context(tc.tile_pool(name="xstr", bufs=2))
            xkstr = sctx.enter_context(tc.tile_pool(name="xkstr", bufs=3))
            vtp = sctx.enter_context(tc.tile_pool(name="vtp", bufs=1))
            ykvp = sctx.enter_context(tc.tile_pool(name="ykvp", bufs=1))
            psS = sctx.enter_context(tc.tile_pool(name="psS", bufs=2,
                                                  space="PSUM"))

            y_q = ykvp.tile([KD, B, NQ], BF16, tag="yq")
            st_q = small.tile([KD, 2 * NQC, 6], F32, tag="st_q")
            for b in range(B):
                xs_c = xstr.tile([128, 2, NQ], BF16, tag="xs")
                nc.sync.dma_start(
                    out=xs_c, in_=xsT[b].rearrange("c p q -> p c q"))
                for t in range(NQC):
                    ps = psS.tile([KD, QC], F32, tag="ps_q")
                    for c in range(2):
                        nc.tensor.matmul(ps, wq_sb[:, c, :],
                                         xs_c[:, c, bass.ts(t, QC)],
                                         start=(c == 0), stop=(c == 1))
                    nc.vector.tensor_copy(y_q[:, b, bass.ts(t, QC)], ps)
                    nc.vector.bn_stats(out=st_q[:, b * NQC + t, :],
                                       in_=y_q[:, b, bass.ts(t, QC)])

            y_kv = ykvp.tile([KVP, B, NPAD], BF16, tag="ykv")
            st_kv = small.tile([KVP, 2 * NT_KV, 6], F32, tag="st_kv")
            XCH = 3 * QC  # 1344-token x stream chunks
            for b in range(B):
                for tt in range(NPAD // XCH):
                    xt_c = xkstr.tile([128, 2, XCH], BF16, tag="xt")
                    nc.sync.dma_start(
                        out=xt_c, in_=xT[b, :, :, bass.ts(tt, XCH)].rearrange(
                            "c p q -> p c q"))
                    for t3 in range(3):
                        t = tt * 3 + t3
                        ps = psS.tile([KVP, QC], F32, tag="ps_kv")
                        for c in range(2):
                            nc.tensor.matmul(ps, wkv_sb[:, c, :],
                                             xt_c[:, c, bass.ts(t3, QC)],
                                             start=(c == 0), stop=(c == 1))
                        nc.vector.tensor_copy(y_kv[:, b, bass.ts(t, QC)], ps)
                        # stats over real tokens only: 5292 = 12*441
                        nc.vector.bn_stats(out=st_kv[:, b * NT_KV + t, :],
                                           in_=y_kv[:, b, bass.ds(t * 441, 441)])

            mv_q = small.tile([KD, 2], F32, tag="mv_q")
            nc.vector.bn_aggr(out=mv_q, in_=st_q)
            s_q, t_q = bn_scale_shift(mv_q, qgb_sb, KD, "q")
            mv_kv = small.tile([KVP, 2], F32, tag="mv_kv")
            nc.vector.bn_aggr(out=mv_kv, in_=st_kv)
            s_kv, t_kv = bn_scale_shift(mv_kv, kvgb_sb, KVP, "kv")

            for b in range(B):
                nc.vector.tensor_scalar(out=kT[:, b, :], in0=y_kv[0:KD, b, :],
                                        scalar1=s_kv[0:KD], scalar2=t_kv[0:KD],
                                        op0=mult, op1=add)
                nc.vector.tensor_scalar(out=qT[:, b, :], in0=y_q[:, b, :],
                                        scalar1=s_q, scalar2=t_q,
                                        op0=mult, op1=add)
            for b in range(B):
                vTn = vtp.tile([DV, NPAD], BF16, tag="vTn")
                nc.vector.tensor_scalar(out=vTn, in0=y_kv[32:KVP, b, :],
                                        scalar1=s_kv[32:KVP],
                                        scalar2=t_kv[32:KVP],
                                        op0=mult, op1=add)
                vtd = vtp.tile([128, KTN, DV], BF16, tag="vtd")
                nc.sync.dma_start_transpose(out=vtd, in_=vTn)
                nc.vector.tensor_copy(v_aug[:, b, :, 0:DV], vtd)
                nc.vector.memset(v_aug[:, b, :, DV:DV + 1], 1.0)

        # ------------------------- attention -------------------------
        asb = ctx.enter_context(tc.tile_pool(name="asb", bufs=1))
        sp0 = asb.tile([128, KTN, QC], BF16, tag="sp0")
        sp1 = asb.tile([128, KTN, QC], BF16, tag="sp1")
        sps = [sp0, sp1]
        hsT = asb.tile([DV, B, NQ], BF16, tag="hsT")
        hs_bounce = dram.tile([NQC, DV, B * QC], BF16, tag="hs_bounce")
        hs_all = dram.tile([NQC, H * DV, B * QC], BF16, tag="hs_all")

        psA = ctx.enter_context(tc.tile_pool(name="psA", bufs=2, space="PSUM"))
        psB = ctx.enter_context(tc.tile_pool(name="psB", bufs=1, space="PSUM"))
        psE = ctx.enter_context(tc.tile_pool(name="psE", bufs=2, space="PSUM"))

        y_p = asb.tile([OC, B * NQ], F32, tag="yp")
        st_p = small.tile([OC, B * NQ // QC, 6], F32, tag="st_p")

        for qc in range(NQC):
            # QK + bias + exp, streaming bias tiles (6 k-tiles + slop each)
            for m in range(NBD):
                b2 = bpool.tile([128, (BDMA + 1) * QC], F8, tag="b2")
                nc.sync.dma_start(out=b2, in_=bT[qc, m])
                b2v = b2.rearrange("p (s q) -> p s q", q=QC)
                for gg in range(BDMA // GRP):
                    for b in range(B):
                        # each 512-aligned region owns one PSUM bank
                        qk = psA.tile([128, GRP, 512], F32, tag="qk")
                        for i in range(GRP):
                            s = gg * GRP + i
                            j = m * BDMA + s
                            nc.tensor.matmul(qk[:, i, 0:QC],
                                             kT[:, b, bass.ts(j, 128)],
                                             qT[:, b, bass.ts(qc, QC)],
                                             start=True, stop=False)
                            nc.tensor.matmul(
                                qk[:, i, 0:QC], i2_sb[:, :, :],
                                b2v[:, s:s + 2, :],
                                start=False, stop=True, perf_mode=DR,
                                skip_group_check=True)
                        g = m * (BDMA // GRP) + gg
                        nc.scalar.activation(
                            out=sps[b][:, bass.ds(g * GRP, GRP), :],
                            in_=qk[:, :, 0:QC], func=Act.Exp)

            # attn@V swapped; region-sequential chains, av owns one bank
            av = psB.tile([QQ, B, NQQ, DV + 1], F32, tag="av")
            tp = psB.tile([DV, B, QC], BF16, tag="tp")
            for b in range(B):
                for qq in range(NQQ):
                    for j in range(KTN):
                        nc.tensor.matmul(av[:, b, qq, :],
                                         sps[b][:, j, bass.ds(qq * QQ, QQ)],
                                         v_aug[:, b, j, :],
                                         start=(j == 0), stop=(j == KTN - 1),
                                         skip_group_check=True)
                # drain: denominators via per-partition scalars + hardswish
                av_sb = drain.tile([QQ, NQQ, DV + 1], F32, tag="av_sb")
                nc.vector.tensor_copy(av_sb, av[:, b, :, :])
                rec = drain.tile([QQ, NQQ, 1], F32, tag="rec")
                nc.vector.reciprocal(out=rec, in_=av_sb[:, :, DV:DV + 1])
                xo = drain.tile([QQ, NQQ, DV], F32, tag="xo")
                for qq in range(NQQ):
                    nc.vector.tensor_scalar(out=xo[:, qq, :],
                                            in0=av_sb[:, qq, 0:DV],
                                            scalar1=rec[:, qq, :],
                                            scalar2=None, op0=mult)
                r3 = drain.tile([QQ, NQQ, DV], F32, tag="r3")
                nc.vector.tensor_scalar(out=r3, in0=xo, scalar1=3.0,
                                        scalar2=0.0, op0=add,
                                        op1=mybir.AluOpType.max)
                nc.vector.tensor_scalar(out=r3, in0=r3, scalar1=6.0,
                                        scalar2=1.0 / 6.0, op0=amin, op1=mult)
                hs_tok = drain.tile([QQ, NQQ, DV], BF16, tag="hs_tok")
                nc.vector.tensor_mul(hs_tok, xo, r3)
                for qq in range(NQQ):
                    nc.tensor.transpose(tp[:, b, bass.ds(qq * QQ, QQ)],
                                        hs_tok[:, qq, :],
                                        ident_sb[0:QQ, 0:QQ])
                nc.vector.tensor_copy(hsT[:, b, bass.ts(qc, QC)], tp[:, b, :])
            nc.sync.dma_start(
                out=hs_bounce[qc].rearrange("d (b q) -> d b q", b=B),
                in_=hsT[:, :, bass.ts(qc, QC)])
            nc.gpsimd.collective_compute(
                "AllGather", mybir.AluOpType.bypass,
                replica_groups=[list(range(NCORES))],
                ins=[hs_bounce[qc].opt()],
                outs=[hs_all[qc].opt()])

            # ---- projection for this chunk (overlaps later attention) ----
            hsall_sb = drain.tile([128, 2, B * QC], BF16, tag="hsall")
            nc.sync.dma_start(
                out=hsall_sb,
                in_=hs_all[qc].rearrange("(c p) t -> p c t", c=2))
            for b in range(B):
                ps = psE.tile([OC, QC], F32, tag="ps_p")
                for c in range(2):
                    nc.tensor.matmul(ps, wp_sb[:, c, :],
                                     hsall_sb[:, c, bass.ds(b * QC, QC)],
                                     start=(c == 0), stop=(c == 1))
                i = b * NQC + qc
                nc.vector.tensor_copy(
                    y_p[:, bass.ds(b * NQ + qc * QC, QC)], ps)
                nc.vector.bn_stats(out=st_p[:, i, :],
                                   in_=y_p[:, bass.ds(b * NQ + qc * QC, QC)])

        # ------------------------- final BN -------------------------
        mv_p = small.tile([OC, 2], F32, tag="mv_p")
        nc.vector.bn_aggr(out=mv_p, in_=st_p)
        s_p, t_p = bn_scale_shift(mv_p, pgb_sb, OC, "p")
        nc.vector.tensor_scalar(out=y_p, in0=y_p, scalar1=s_p, scalar2=t_p,
                                op0=mult, op1=add)
        nc.sync.dma_start(out=yT[:, :], in_=y_p)

    nc.finalize()
    return nc


def _prep_inputs(x, kv_w, kv_g, kv_b, q_w, q_g, q_b, proj_w, proj_g, proj_b,
                 bias_table, bias_idxs):
    """Host-side sharding/layout prep. Returns list of 8 per-core input maps."""
    x = np.asarray(x, np.float32)
    xt = np.zeros((B, 2, 128, NPAD), np.float32)
    xTt = x.transpose(0, 2, 1)  # (B, 256, N)
    xt[:, :, :, :N] = xTt.reshape(B, 2, 128, N)
    xt = xt.astype(bf16)
    xs = x.reshape(B, ROW, COL, CIN)[:, ::2, ::2].reshape(B, NQ, CIN)
    xst = xs.transpose(0, 2, 1).reshape(B, 2, 128, NQ).astype(bf16)

    # raw bias (not exp), padded-k rows get -32 (dead after exp)
    rank2 = np.asarray(bias_idxs)[0].reshape(ROW, COL)
    table2 = np.asarray(bias_table, np.float32)[:, rank2]  # (H, 63, 84)
    kk = np.arange(N)
    qq = np.arange(NQ)
    DRm = np.abs(kk[:, None] // COL - 2 * (qq[None, :] // COL_))
    DCm = np.abs(kk[:, None] % COL - 2 * (qq[None, :] % COL_))

    i2 = np.zeros((128, 2, 128), np.float32)
    i2[:, 0, :] = np.eye(128)
    i2 = i2.astype(f8e4)
    ident = np.eye(128, dtype=np.float32).astype(bf16)

    in_maps = []
    for h in range(H):
        bfull = np.full((NPAD, NQ), -32.0, np.float32)
        bfull[:N] = table2[h][DRm, DCm]
        # (NPAD, NQ) -> (NQC, NBD, 128, (BDMA+1)*QC), slot s = k-tile
        # BDMA*m + s, slot BDMA = next DMA-chunk's first tile (slop)
        bk = bfull.reshape(KTN, 128, NQC, QC)       # (ktile, p, qc, q)
        bl = np.zeros((NQC, NBD, 128, (BDMA + 1) * QC), np.float32)
        for m in range(NBD):
            for s in range(BDMA + 1):
                j = m * BDMA + s
                if j < KTN:
                    bl[:, m, :, s * QC:(s + 1) * QC] = bk[j].transpose(1, 0, 2)
        blf = bl.astype(f8e4)

        sl = slice(h * HKV, (h + 1) * HKV)
        slq = slice(h * KD, (h + 1) * KD)
        slo = slice(h * OC, (h + 1) * OC)
        wkv_pad = np.zeros((KVP, CIN), np.float32)
        wkv_pad[0:KD] = np.asarray(kv_w, np.float32)[sl][0:KD]
        wkv_pad[32:KVP] = np.asarray(kv_w, np.float32)[sl][KD:HKV]
        # packed weights: [kv(64) | q(16) | proj(64)] columns
        w_all = np.concatenate([
            wkv_pad,
            np.asarray(q_w, np.float32)[slq],
            np.asarray(proj_w, np.float32)[slo],
        ], axis=0)  # (144, 256)
        # packed gains/biases, column groups kv|q|p; SCALE folded into q
        gb_all = np.zeros((KVP, 6), np.float32)
        gb_all[:, 0] = 1.0
        gb_all[0:KD, 0] = np.asarray(kv_g, np.float32)[sl][0:KD]
        gb_all[0:KD, 1] = np.asarray(kv_b, np.float32)[sl][0:KD]
        gb_all[32:KVP, 0] = np.asarray(kv_g, np.float32)[sl][KD:HKV]
        gb_all[32:KVP, 1] = np.asarray(kv_b, np.float32)[sl][KD:HKV]
        gb_all[0:KD, 2] = np.asarray(q_g, np.float32)[slq] * SCALE
        gb_all[0:KD, 3] = np.asarray(q_b, np.float32)[slq] * SCALE
        gb_all[0:OC, 4] = np.asarray(proj_g, np.float32)[slo]
        gb_all[0:OC, 5] = np.asarray(proj_b, np.float32)[slo]
        in_maps.append({
            "xT": xt,
            "xsT": xst,
            "wT": np.ascontiguousarray(
                w_all.T.reshape(2, 128, WTOT)).astype(bf16),
            "gbT": np.ascontiguousarray(gb_all),
            "i2T": i2,
            "identT": ident,
            "bT": blf,
        })
    return in_maps


def kernel(x, kv_w, kv_g, kv_b, q_w, q_g, q_b, proj_w, proj_g, proj_b,
           bias_table, bias_idxs, _trace=False):
    global LAST_EXEC_NS
    if "nc" not in _prog_cache:
        _prog_cache["nc"] = _build_program()
    nc = _prog_cache["nc"]
    in_maps = _prep_inputs(x, kv_w, kv_g, kv_b, q_w, q_g, q_b,
                           proj_w, proj_g, proj_b, bias_table, bias_idxs)
    res = run_bass_kernel_spmd(nc, in_maps, core_ids=list(range(NCORES)),
                               trace=_trace)
    LAST_EXEC_NS = res.exec_time_ns
    yts = [np.asarray(r["yT"]) for r in res.results]  # each (OC, B*NQ)
    y = np.concatenate(yts, axis=0)                   # (512, B*NQ)
    return np.ascontiguousarray(
        y.T.reshape(B, NQ, H * OC).astype(np.float32))


# revision 17
# speedup vs baseline: 1.0611x; 1.0611x over previous
"""AttentionSubsample kernel for 8 trn2 NeuronCores (v2.2).

Sharding: head-parallel (8 heads -> 8 cores); final projection sharded by
output channels after an AllGather of per-head attention outputs.

Cost-model-driven design:
- Relative-position bias added INTO the QK PSUM by fp8e4m3 DoubleRow
  matmuls (identity stationary, bias moving); softmax scale folded into
  the q BN affine host-side; ACT does exp only.
- attn@V swapped (attention stationary, V moving, moving size 33);
  attention strips stay in SBUF (3 rotating buffers); 42-tile PSUM
  accumulation chains run region-sequential (a start=True clears
  has_written for its whole bank; qk regions are 512-aligned).
- Software-pipelined emission: engine queues and the DMA queue are
  strictly in-order, so every wait is placed where its producer is
  already done: bias DMAs stream contiguously; the AllGather-dependent
  hsall load + projection of chunk n-1 are emitted inside chunk n;
  attn@V chains of chunk n-1 are emitted after the first bias block of
  chunk n.
- BN stats taken directly from PSUM (DVE) with analytic pad correction;
  psum->SBUF copies in the head run on the then-idle ACT engine.
"""

import numpy as np
import ml_dtypes

import concourse.bass as bass
import concourse.mybir as mybir
import concourse.tile as tile
from concourse import bacc
from contextlib import ExitStack
from concourse.bass_utils import run_bass_kernel_spmd

BF16 = mybir.dt.bfloat16
F32 = mybir.dt.float32
F8 = mybir.dt.float8e4
bf16 = ml_dtypes.bfloat16
f8e4 = ml_dtypes.float8_e4m3

B = 2
ROW, COL = 63, 84
ROW_, COL_ = 32, 42
N = ROW * COL            # 5292 kv tokens
NQ = ROW_ * COL_         # 1344 q tokens
NPAD = 5376              # 42*128 padded kv tokens
KTN = NPAD // 128        # 42 k-tiles
QC = 448                 # q chunk
NQC = NQ // QC           # 3
QQ = 112                 # q sub-chunk (attn@V stationary width)
NQQ = QC // QQ           # 4
CIN = 256
H = 8
KD = 16
DV = 32
HKV = KD + DV            # 48 per-head kv channels
KVP = 64                 # padded kv rows: k at 0:16, v at 32:64
OC = 64                  # per-core slice of the 512 output channels
GRP = 2                  # k-tiles per exp group
NGRP = KTN // GRP        # 21
BDMA = 6                 # k-tiles per bias DMA (+1 slop)
NBD = KTN // BDMA        # 7 bias DMAs per q chunk
WTOT = KVP + KD + OC     # 144 packed weight columns
EPS = 1e-5
SCALE = KD ** -0.5
NCORES = 8
PADC = NPAD / N          # pad-inclusion correction for kv stats

LAST_EXEC_NS = None
_prog_cache = {}


def _build_program():
    nc = bacc.Bacc(num_devices=NCORES)

    xT = nc.dram_tensor("xT", [B, 2, 128, NPAD], BF16, kind="ExternalInput")
    xsT = nc.dram_tensor("xsT", [B, 2, 128, NQ], BF16, kind="ExternalInput")
    wT = nc.dram_tensor("wT", [2, 128, WTOT], BF16, kind="ExternalInput")
    gbT = nc.dram_tensor("gbT", [KVP, 6], F32, kind="ExternalInput")
    i2T = nc.dram_tensor("i2T", [128, 2, 128], F8, kind="ExternalInput")
    identT = nc.dram_tensor("identT", [128, 128], BF16, kind="ExternalInput")
    bT = nc.dram_tensor("bT", [NQC, NBD, 128, (BDMA + 1) * QC], F8,
                        kind="ExternalInput")
    yT = nc.dram_tensor("yT", [OC, B * NQ], F32, kind="ExternalOutput")

    with ExitStack() as ctx:
        tc = ctx.enter_context(tile.TileContext(nc))
        const = ctx.enter_context(tc.tile_pool(name="const", bufs=1))
        big = ctx.enter_context(tc.tile_pool(name="big", bufs=1))
        bpool = ctx.enter_context(tc.tile_pool(name="bpool", bufs=8))
        small = ctx.enter_context(tc.tile_pool(name="small", bufs=4))
        drain = ctx.enter_context(tc.tile_pool(name="drain", bufs=2))
        dram = ctx.enter_context(tc.tile_pool(name="dram", bufs=4, space="DRAM"))

        mult = mybir.AluOpType.mult
        add = mybir.AluOpType.add
        amin = mybir.AluOpType.min
        Act = mybir.ActivationFunctionType
        DR = mybir.MatmulPerfMode.DoubleRow

        # ------------------------- consts -------------------------
        w_sb = const.tile([128, 2, WTOT], BF16, tag="w")
        for c in range(2):
            nc.sync.dma_start(out=w_sb[:, c, :], in_=wT[c])
        gb_sb = const.tile([KVP, 6], F32, tag="gb")
        nc.sync.dma_start(out=gb_sb, in_=gbT[:, :])
        i2_sb = const.tile([128, 2, 128], F8, tag="i2")
        nc.sync.dma_start(out=i2_sb, in_=i2T[:, :, :])
        ident_sb = const.tile([128, 128], BF16, tag="ident")
        nc.sync.dma_start(out=ident_sb, in_=identT[:, :])
        eps_t = const.tile([128, 1], F32, tag="eps")
        nc.vector.memset(eps_t, EPS)
        wkv_sb = w_sb[:, :, 0:KVP]
        wq_sb = w_sb[:, :, KVP:KVP + KD]
        wp_sb = w_sb[:, :, KVP + KD:WTOT]
        kvgb_sb = gb_sb[:, 0:2]
        qgb_sb = gb_sb[0:KD, 2:4]
        pgb_sb = gb_sb[0:OC, 4:6]

        # persistent normalized tensors
        kT = big.tile([KD, B, NPAD], BF16, tag="kT")
        qT = big.tile([KD, B, NQ], BF16, tag="qT")
        v_aug = big.tile([128, B, KTN, DV + 1], BF16, tag="vaug")

        def bn_scale_shift(mv, gb, P, name):
            s = small.tile([P, 1], F32, tag=f"s_{name}", name=f"s_{name}")
            t = small.tile([P, 1], F32, tag=f"t_{name}", name=f"t_{name}")
            nc.scalar.activation(out=s, in_=mv[:, 1:2], func=Act.Sqrt,
                                 bias=eps_t[0:P])
            nc.vector.reciprocal(out=s, in_=s)
            nc.vector.tensor_mul(s, s, gb[:, 0:1])
            nc.vector.tensor_mul(t, mv[:, 0:1], s)
            nc.vector.tensor_scalar(out=t, in0=t, scalar1=-1.0, scalar2=None,
                                    op0=mult)
            nc.vector.tensor_add(t, t, gb[:, 1:2])
            return s, t

        # ------------------- start phase (scoped pools) -------------------
        NT_KV = NPAD // QC   # 12
        with ExitStack() as sctx:
            xstr = sctx.enter_context(tc.tile_pool(name="xstr", bufs=2))
            xkstr = sctx.enter_context(tc.tile_pool(name="xkstr", bufs=3))
            vtp = sctx.enter_context(tc.tile_pool(name="vtp", bufs=1))
            ykvp = sctx.enter_context(tc.tile_pool(name="ykvp", bufs=1))
            psS = sctx.enter_context(tc.tile_pool(name="psS", bufs=2,
                                                  space="PSUM"))

            y_q = ykvp.tile([KD, B, NQ], BF16, tag="yq")
            st_q = small.tile([KD, 2 * NQC, 6], F32, tag="st_q")
            for b in range(B):
                xs_c = xstr.tile([128, 2, NQ], BF16, tag="xs")
                nc.sync.dma_start(
                    out=xs_c, in_=xsT[b].rearrange("c p q -> p c q"))
                for t in range(NQC):
                    ps = psS.tile([KD, QC], F32, tag="ps_q")
                    for c in range(2):
                        nc.tensor.matmul(ps, wq_sb[:, c, :],
                                         xs_c[:, c, bass.ts(t, QC)],
                                         start=(c == 0), stop=(c == 1))
                    # stats straight from PSUM; sbuf copy on idle ACT
                    nc.vector.bn_stats(out=st_q[:, b * NQC + t, :], in_=ps)
                    nc.scalar.copy(y_q[:, b, bass.ts(t, QC)], ps)

            y_kv = ykvp.tile([KVP, B, NPAD], BF16, tag="ykv")
            st_kv = small.tile([KVP, 2 * NT_KV, 6], F32, tag="st_kv")
            XCH = 3 * QC  # 1344-token x stream chunks
            for b in range(B):
                for tt in range(NPAD // XCH):
                    xt_c = xkstr.tile([128, 2, XCH], BF16, tag="xt")
                    nc.sync.dma_start(
                        out=xt_c, in_=xT[b, :, :, bass.ts(tt, XCH)].rearrange(
                            "c p q -> p c q"))
                    for t3 in range(3):
                        t = tt * 3 + t3
                        ps = psS.tile([KVP, QC], F32, tag="ps_kv")
                        for c in range(2):
                            nc.tensor.matmul(ps, wkv_sb[:, c, :],
                                             xt_c[:, c, bass.ts(t3, QC)],
                                             start=(c == 0), stop=(c == 1))
                        # padded-token zeros included; corrected after aggr
                        nc.vector.bn_stats(out=st_kv[:, b * NT_KV + t, :],
                                           in_=ps)
                        nc.scalar.copy(y_kv[:, b, bass.ts(t, QC)], ps)

            mv_q = small.tile([KD, 2], F32, tag="mv_q")
            nc.vector.bn_aggr(out=mv_q, in_=st_q)
            s_q, t_q = bn_scale_shift(mv_q, qgb_sb, KD, "q")

            # kv stats included 2*84 pad zeros: correct mean/var analytically
            mv_kv0 = small.tile([KVP, 2], F32, tag="mv_kv0")
            nc.vector.bn_aggr(out=mv_kv0, in_=st_kv)
            mv_kv = small.tile([KVP, 2], F32, tag="mv_kv")
            sq_kv = small.tile([KVP, 1], F32, tag="sq_kv")
            # e2 = (var' + mean'^2) * c ; mean = mean' * c ; var = e2 - mean^2
            nc.vector.tensor_mul(sq_kv, mv_kv0[:, 0:1], mv_kv0[:, 0:1])
            nc.vector.tensor_add(sq_kv, sq_kv, mv_kv0[:, 1:2])
            nc.vector.tensor_scalar(out=sq_kv, in0=sq_kv, scalar1=PADC,
                                    scalar2=None, op0=mult)
            nc.vector.tensor_scalar(out=mv_kv[:, 0:1], in0=mv_kv0[:, 0:1],
                                    scalar1=PADC, scalar2=None, op0=mult)
            nc.vector.tensor_mul(mv_kv[:, 1:2], mv_kv[:, 0:1], mv_kv[:, 0:1])
            nc.vector.tensor_sub(mv_kv[:, 1:2], sq_kv, mv_kv[:, 1:2])
            s_kv, t_kv = bn_scale_shift(mv_kv, kvgb_sb, KVP, "kv")

            for b in range(B):
                nc.vector.tensor_scalar(out=kT[:, b, :], in0=y_kv[0:KD, b, :],
                                        scalar1=s_kv[0:KD], scalar2=t_kv[0:KD],
                                        op0=mult, op1=add)
                nc.vector.tensor_scalar(out=qT[:, b, :], in0=y_q[:, b, :],
                                        scalar1=s_q, scalar2=t_q,
                                        op0=mult, op1=add)
            for b in range(B):
                vTn = vtp.tile([DV, NPAD], BF16, tag="vTn")
                nc.vector.tensor_scalar(out=vTn, in0=y_kv[32:KVP, b, :],
                                        scalar1=s_kv[32:KVP],
                                        scalar2=t_kv[32:KVP],
                                        op0=mult, op1=add)
                vtd = vtp.tile([128, KTN, DV], BF16, tag="vtd")
                # ACT hwdge queue: keeps the SP queue free for bias tiles
                nc.scalar.dma_start_transpose(out=vtd, in_=vTn)
                nc.vector.tensor_copy(v_aug[:, b, :, 0:DV], vtd)
                nc.vector.memset(v_aug[:, b, :, DV:DV + 1], 1.0)

        # ------------------------- attention -------------------------
        asb = ctx.enter_context(tc.tile_pool(name="asb", bufs=1))
        sps = [asb.tile([128, KTN, QC], BF16, tag=f"sp{i}", name=f"sp{i}")
               for i in range(3)]
        hsT = asb.tile([DV, B, NQ], BF16, tag="hsT")
        y_p = asb.tile([OC, B * NQ], F32, tag="yp")
        st_p = small.tile([OC, B * NQ // QC, 6], F32, tag="st_p")
        hs_bounce = dram.tile([NQC, DV, B * QC], BF16, tag="hs_bounce")
        hs_all = dram.tile([NQC, H * DV, B * QC], BF16, tag="hs_all")

        psA = ctx.enter_context(tc.tile_pool(name="psA", bufs=2, space="PSUM"))
        psB = ctx.enter_context(tc.tile_pool(name="psB", bufs=1, space="PSUM"))
        psE = ctx.enter_context(tc.tile_pool(name="psE", bufs=2, space="PSUM"))

        def strip(qc, b):
            return sps[(qc * B + b) % 3]

        def emit_bias_dmas(qc):
            """Prefetch all 7 bias tiles for a chunk (in-order DMA stream)."""
            tiles = []
            for m in range(NBD):
                b2 = bpool.tile([128, (BDMA + 1) * QC], F8, tag="b2")
                nc.sync.dma_start(out=b2, in_=bT[qc, m])
                tiles.append(b2.rearrange("p (s q) -> p s q", q=QC))
            return tiles

        def emit_bias_block(qc, m, b, b2v):
            """QK+bias+exp for one batch's 3 groups of bias tile m."""
            for gg in range(BDMA // GRP):
                qk = psA.tile([128, GRP, 512], F32, tag="qk")
                for i in range(GRP):
                    s = gg * GRP + i
                    j = m * BDMA + s
                    nc.tensor.matmul(qk[:, i, 0:QC],
                                     kT[:, b, bass.ts(j, 128)],
                                     qT[:, b, bass.ts(qc, QC)],
                                     start=True, stop=False)
                    nc.tensor.matmul(
                        qk[:, i, 0:QC], i2_sb[:, :, :],
                        b2v[:, s:s + 2, :],
                        start=False, stop=True, perf_mode=DR,
                        skip_group_check=True)
                g = m * (BDMA // GRP) + gg
                nc.scalar.activation(
                    out=strip(qc, b)[:, bass.ds(g * GRP, GRP), :],
                    in_=qk[:, :, 0:QC], func=Act.Exp)

        av_tiles = {}

        def emit_chains(qc, b):
            """attn@V accumulation chains (region-sequential, one bank)."""
            if qc not in av_tiles:
                av_tiles[qc] = psB.tile([QQ, B, NQQ, DV + 1], F32, tag="av",
                                        name=f"av{qc}")
            av = av_tiles[qc]
            for qq in range(NQQ):
                for j in range(KTN):
                    nc.tensor.matmul(av[:, b, qq, :],
                                     strip(qc, b)[:, j,
                                                  bass.ds(qq * QQ, QQ)],
                                     v_aug[:, b, j, :],
                                     start=(j == 0), stop=(j == KTN - 1),
                                     skip_group_check=True)

        hstoks = {}

        def emit_drain_math(qc, b):
            """denominators + hardswish on DVE (no PE work)."""
            av = av_tiles[qc]
            av_sb = drain.tile([QQ, NQQ, DV + 1], F32, tag="av_sb")
            nc.vector.tensor_copy(av_sb, av[:, b, :, :])
            rec = drain.tile([QQ, NQQ, 1], F32, tag="rec")
            nc.vector.reciprocal(out=rec, in_=av_sb[:, :, DV:DV + 1])
            xo = drain.tile([QQ, NQQ, DV], F32, tag="xo")
            for qq in range(NQQ):
                nc.vector.tensor_scalar(out=xo[:, qq, :],
                                        in0=av_sb[:, qq, 0:DV],
                                        scalar1=rec[:, qq, :],
                                        scalar2=None, op0=mult)
            r3 = drain.tile([QQ, NQQ, DV], F32, tag="r3")
            nc.vector.tensor_scalar(out=r3, in0=xo, scalar1=3.0,
                                    scalar2=0.0, op0=add,
                                    op1=mybir.AluOpType.max)
            nc.vector.tensor_scalar(out=r3, in0=r3, scalar1=6.0,
                                    scalar2=1.0 / 6.0, op0=amin, op1=mult)
            hs_tok = drain.tile([QQ, NQQ, DV], BF16, tag="hs_tok")
            nc.vector.tensor_mul(hs_tok, xo, r3)
            hstoks[(qc, b)] = hs_tok

        tp_tiles = {}

        def emit_drain_tp(qc, b):
            """PE transposes to channel-major + hsT copy."""
            if qc not in tp_tiles:
                tp_tiles[qc] = psB.tile([DV, B, QC], BF16, tag="tp",
                                        name=f"tp{qc}")
            tp = tp_tiles[qc]
            hs_tok = hstoks.pop((qc, b))
            for qq in range(NQQ):
                nc.tensor.transpose(tp[:, b, bass.ds(qq * QQ, QQ)],
                                    hs_tok[:, qq, :],
                                    ident_sb[0:QQ, 0:QQ])
            nc.vector.tensor_copy(hsT[:, b, bass.ts(qc, QC)], tp[:, b, :])

        def emit_gather(qc):
            nc.sync.dma_start(
                out=hs_bounce[qc].rearrange("d (b q) -> d b q", b=B),
                in_=hsT[:, :, bass.ts(qc, QC)])
            nc.gpsimd.collective_compute(
                "AllGather", mybir.AluOpType.bypass,
                replica_groups=[list(range(NCORES))],
                ins=[hs_bounce[qc].opt()],
                outs=[hs_all[qc].opt()])

        def emit_proj(qc):
            """hsall load + projection + partial BN stats for chunk qc."""
            hsall_sb = drain.tile([128, 2, B * QC], BF16, tag="hsall")
            nc.sync.dma_start(
                out=hsall_sb,
                in_=hs_all[qc].rearrange("(c p) t -> p c t", c=2))
            for b in range(B):
                ps = psE.tile([OC, QC], F32, tag="ps_p")
                for c in range(2):
                    nc.tensor.matmul(ps, wp_sb[:, c, :],
                                     hsall_sb[:, c, bass.ds(b * QC, QC)],
                                     start=(c == 0), stop=(c == 1))
                i = b * NQC + qc
                nc.vector.bn_stats(out=st_p[:, i, :], in_=ps)
                nc.vector.tensor_copy(
                    y_p[:, bass.ds(b * NQ + qc * QC, QC)], ps)

        # software-pipelined attention. Program order IS the dependency
        # order for the Tile tracker, so chains(qc-1, b) are emitted before
        # any exp that overwrites their strip (reuse distance 3 with the
        # b0/b1-phased emission below), and every DMA is emitted only after
        # its producer is already in flight.
        for qc in range(NQC):
            b2s = emit_bias_dmas(qc)
            for m in range(NBD):
                emit_bias_block(qc, m, 0, b2s[m])
                if qc > 0:
                    if m == 0:
                        emit_chains(qc - 1, 0)
                        emit_drain_math(qc - 1, 0)
                    if m == 1:
                        emit_drain_tp(qc - 1, 0)
                if m == 3 and qc > 1:
                    emit_proj(qc - 2)
            for m in range(NBD):
                emit_bias_block(qc, m, 1, b2s[m])
                if qc > 0:
                    if m == 0:
                        emit_chains(qc - 1, 1)
                        emit_drain_math(qc - 1, 1)
                    if m == 1:
                        emit_drain_tp(qc - 1, 1)
                        emit_gather(qc - 1)
        for b in range(B):
            emit_chains(NQC - 1, b)
            emit_drain_math(NQC - 1, b)
            emit_drain_tp(NQC - 1, b)
        emit_gather(NQC - 1)
        emit_proj(NQC - 2)
        emit_proj(NQC - 1)

        # ------------------------- final BN -------------------------
        mv_p = small.tile([OC, 2], F32, tag="mv_p")
        nc.vector.bn_aggr(out=mv_p, in_=st_p)
        s_p, t_p = bn_scale_shift(mv_p, pgb_sb, OC, "p")
        nc.vector.tensor_scalar(out=y_p, in0=y_p, scalar1=s_p, scalar2=t_p,
                                op0=mult, op1=add)
        nc.sync.dma_start(out=yT[:, :], in_=y_p)

    nc.finalize()
    return nc


def _prep_inputs(x, kv_w, kv_g, kv_b, q_w, q_g, q_b, proj_w, proj_g, proj_b,
                 bias_table, bias_idxs):
    """Host-side sharding/layout prep. Returns list of 8 per-core input maps."""
    x = np.asarray(x, np.float32)
    xt = np.zeros((B, 2, 128, NPAD), np.float32)
    xTt = x.transpose(0, 2, 1)  # (B, 256, N)
    xt[:, :, :, :N] = xTt.reshape(B, 2, 128, N)
    xt = xt.astype(bf16)
    xs = x.reshape(B, ROW, COL, CIN)[:, ::2, ::2].reshape(B, NQ, CIN)
    xst = xs.transpose(0, 2, 1).reshape(B, 2, 128, NQ).astype(bf16)

    # raw bias (not exp), padded-k rows get -32 (dead after exp)
    rank2 = np.asarray(bias_idxs)[0].reshape(ROW, COL)
    table2 = np.asarray(bias_table, np.float32)[:, rank2]  # (H, 63, 84)
    kk = np.arange(N)
    qq = np.arange(NQ)
    DRm = np.abs(kk[:, None] // COL - 2 * (qq[None, :] // COL_))
    DCm = np.abs(kk[:, None] % COL - 2 * (qq[None, :] % COL_))

    i2 = np.zeros((128, 2, 128), np.float32)
    i2[:, 0, :] = np.eye(128)
    i2 = i2.astype(f8e4)
    ident = np.eye(128, dtype=np.float32).astype(bf16)

    in_maps = []
    for h in range(H):
        bfull = np.full((NPAD, NQ), -32.0, np.float32)
        bfull[:N] = table2[h][DRm, DCm]
        # (NPAD, NQ) -> (NQC, NBD, 128, (BDMA+1)*QC), slot s = k-tile
        # BDMA*m + s, slot BDMA = next DMA-chunk's first tile (slop)
        bk = bfull.reshape(KTN, 128, NQC, QC)       # (ktile, p, qc, q)
        bl = np.zeros((NQC, NBD, 128, (BDMA + 1) * QC), np.float32)
        for m in range(NBD):
            for s in range(BDMA + 1):
                j = m * BDMA + s
                if j < KTN:
                    bl[:, m, :, s * QC:(s + 1) * QC] = bk[j].transpose(1, 0, 2)
        blf = bl.astype(f8e4)

        sl = slice(h * HKV, (h + 1) * HKV)
        slq = slice(h * KD, (h + 1) * KD)
        slo = slice(h * OC, (h + 1) * OC)
        wkv_pad = np.zeros((KVP, CIN), np.float32)
        wkv_pad[0:KD] = np.asarray(kv_w, np.float32)[sl][0:KD]
        wkv_pad[32:KVP] = np.asarray(kv_w, np.float32)[sl][KD:HKV]
        # packed weights: [kv(64) | q(16) | proj(64)] columns
        w_all = np.concatenate([
            wkv_pad,
            np.asarray(q_w, np.float32)[slq],
            np.asarray(proj_w, np.float32)[slo],
        ], axis=0)  # (144, 256)
        # packed gains/biases, column groups kv|q|p; SCALE folded into q
        gb_all = np.zeros((KVP, 6), np.float32)
        gb_all[:, 0] = 1.0
        gb_all[0:KD, 0] = np.asarray(kv_g, np.float32)[sl][0:KD]
        gb_all[0:KD, 1] = np.asarray(kv_b, np.float32)[sl][0:KD]
        gb_all[32:KVP, 0] = np.asarray(kv_g, np.float32)[sl][KD:HKV]
        gb_all[32:KVP, 1] = np.asarray(kv_b, np.float32)[sl][KD:HKV]
        gb_all[0:KD, 2] = np.asarray(q_g, np.float32)[slq] * SCALE
        gb_all[0:KD, 3] = np.asarray(q_b, np.float32)[slq] * SCALE
        gb_all[0:OC, 4] = np.asarray(proj_g, np.float32)[slo]
        gb_all[0:OC, 5] = np.asarray(proj_b, np.float32)[slo]
        in_maps.append({
            "xT": xt,
            "xsT": xst,
            "wT": np.ascontiguousarray(
                w_all.T.reshape(2, 128, WTOT)).astype(bf16),
            "gbT": np.ascontiguousarray(gb_all),
            "i2T": i2,
            "identT": ident,
            "bT": blf,
        })
    return in_maps


def kernel(x, kv_w, kv_g, kv_b, q_w, q_g, q_b, proj_w, proj_g, proj_b,
           bias_table, bias_idxs, _trace=False):
    global LAST_EXEC_NS
    if "nc" not in _prog_cache:
        _prog_cache["nc"] = _build_program()
    nc = _prog_cache["nc"]
    in_maps = _prep_inputs(x, kv_w, kv_g, kv_b, q_w, q_g, q_b,
                           proj_w, proj_g, proj_b, bias_table, bias_idxs)
    res = run_bass_kernel_spmd(nc, in_maps, core_ids=list(range(NCORES)),
                               trace=_trace)
    LAST_EXEC_NS = res.exec_time_ns
    yts = [np.asarray(r["yT"]) for r in res.results]  # each (OC, B*NQ)
    y = np.concatenate(yts, axis=0)                   # (512, B*NQ)
    return np.ascontiguousarray(
        y.T.reshape(B, NQ, H * OC).astype(np.float32))
